# revision 5
# baseline (speedup 1.0000x reference)
"""BEiT attention block kernel for 8 Trainium2 NeuronCores.

Strategy: data-parallel over batch B=256 -> 32 items per core. Weights and the
(gathered, exponentiated, transposed) relative-position bias table are
replicated to every core. All matmuls run in bf16 with fp32 PSUM accumulation.

Per-core pipeline (item pair at a time):
  x [197,768] -> cast bf16 -> PE transpose -> xT [768(d), 197(tok)]
  qT/kT = Wqk^T-stationary matmuls -> [1536(o), 394(tok of 2 items)]
          (q rows get scale*psum + scale*q_bias via ACT)
  v     = xT-stationary matmuls -> natural layout [tok, 768], augmented with a
          ones column per head -> v_aug [tok, head, 65]
  S^T   = kT_h.T @ qT_h per (head, key-chunk)  [nk, nq] in PSUM
  Pexp  = exp(S^T) * exp(biasT)   (bias folded in as a bf16 multiply)
  O^T|sums = v_aug.T @ Pexp  -> [65, nq]; row 64 = softmax denominators
  normalize rows 0..63 by broadcast 1/sums, reorder to OT [768(d), 197(tok)]
  out   = OT-stationary matmuls against proj_w^T -> [tok, 768] + proj_b -> DRAM
"""

import os
import sys
from contextlib import ExitStack

for _p in ("/opt/trn_rl_repo", "/opt/pypackages"):
    if os.path.isdir(_p) and _p not in sys.path:
        sys.path.append(_p)

import numpy as np
import ml_dtypes

import concourse.bacc as bacc
import concourse.bass as bass
import concourse.mybir as mybir
import concourse.tile as tile

BF16 = ml_dtypes.bfloat16

N_CORES = 8
B = 256
BC = B // N_CORES          # items per core
N = 197                    # tokens
D = 768
H = 12
DH = 64
DC = D // 128              # 6 d-chunks of 128
NQK = 2 * D                # q+k output rows
SCALE = DH ** -0.5
CH0, CH1 = 128, N - 128    # token chunks (128, 69)
CHUNKS = ((0, CH0), (CH0, CH1))
VS = 384                   # v / proj free-dim slice (2 slices of 384 = 768)


def _build_body(ctx, tc, t, n_items):
    nc = tc.nc
    f32 = mybir.dt.float32
    bf16 = mybir.dt.bfloat16
    Ident = mybir.ActivationFunctionType.Identity
    Exp = mybir.ActivationFunctionType.Exp
    ADD = mybir.AluOpType.add

    const = ctx.enter_context(tc.tile_pool(name="const", bufs=1))
    wqk = const.tile([128, DC, NQK], bf16)
    nc.sync.dma_start(wqk[:], t["wqk"])
    wv = const.tile([128, DC, D], bf16)
    nc.sync.dma_start(wv[:], t["wv"])
    wp = const.tile([128, DC, D], bf16)
    nc.sync.dma_start(wp[:], t["wp"])
    qb = const.tile([128, DC], f32)
    nc.sync.dma_start(qb[:], t["qb"])
    vb = const.tile([128, D], f32)
    nc.sync.dma_start(vb[:], t["vb"])
    pb = const.tile([128, D], f32)
    nc.sync.dma_start(pb[:], t["pb"])
    eb0 = const.tile([CH0, H, N], bf16)
    nc.sync.dma_start(eb0[:], t["eb0"])
    eb1 = const.tile([CH1, H, N], bf16)
    nc.sync.dma_start(eb1[:], t["eb1"])
    idn = const.tile([128, 128], bf16)
    nc.sync.dma_start(idn[:], t["idn"])

    xin = ctx.enter_context(tc.tile_pool(name="xin", bufs=3))
    xbfp = ctx.enter_context(tc.tile_pool(name="xbf", bufs=3))
    xtp = ctx.enter_context(tc.tile_pool(name="xt", bufs=2))
    qkp = ctx.enter_context(tc.tile_pool(name="qk", bufs=2))
    vap = ctx.enter_context(tc.tile_pool(name="va", bufs=2))
    pep = ctx.enter_context(tc.tile_pool(name="pex", bufs=4))
    otnp = ctx.enter_context(tc.tile_pool(name="otn", bufs=2))
    rcpp = ctx.enter_context(tc.tile_pool(name="rcp", bufs=2))
    otp = ctx.enter_context(tc.tile_pool(name="ot", bufs=2))
    outp = ctx.enter_context(tc.tile_pool(name="outp", bufs=3))
    dramp = ctx.enter_context(tc.tile_pool(name="dram", bufs=2, space="DRAM"))

    ps_tr = ctx.enter_context(tc.tile_pool(name="ps_tr", bufs=1, space="PSUM"))
    ps_qk = ctx.enter_context(tc.tile_pool(name="ps_qk", bufs=2, space="PSUM"))
    ps_v = ctx.enter_context(tc.tile_pool(name="ps_v", bufs=1, space="PSUM"))
    ps_s = ctx.enter_context(tc.tile_pool(name="ps_s", bufs=2, space="PSUM"))
    ps_ot = ctx.enter_context(tc.tile_pool(name="ps_ot", bufs=1, space="PSUM"))
    ps_pr = ctx.enter_context(tc.tile_pool(name="ps_pr", bufs=1, space="PSUM"))

    assert n_items % 2 == 0
    for g in range(n_items // 2):
        xt_g = xtp.tile([128, DC, 2 * N], bf16, tag="xt")
        qkt = qkp.tile([128, 2 * DC, 2 * N], bf16, tag="qkt")

        # ---- load x, cast to bf16, transpose to xT ----
        for ii in range(2):
            item = 2 * g + ii
            for p0, pr in CHUNKS:
                xf = xin.tile([128, D], f32, tag="xf")
                nc.sync.dma_start(xf[0:pr], t["x"][item, p0:p0 + pr, :])
                xb = xbfp.tile([128, D], bf16, tag="xb")
                nc.vector.tensor_copy(xb[0:pr], xf[0:pr])
                for dc in range(DC):
                    pt = ps_tr.tile([128, 128], bf16, tag="tr")
                    nc.tensor.transpose(
                        pt[0:128, 0:pr],
                        xb[0:pr, 128 * dc:128 * (dc + 1)],
                        idn[0:pr, 0:pr],
                    )
                    nc.vector.tensor_copy(
                        xt_g[:, dc, N * ii + p0:N * ii + p0 + pr], pt[:, 0:pr]
                    )

        # ---- qT / kT for the pair: [o-chunk, 394] each ----
        for oc in range(2 * DC):
            pq = ps_qk.tile([128, 2 * N], f32, tag="qkps")
            for dc in range(DC):
                nc.tensor.matmul(
                    pq[:],
                    wqk[:, dc, 128 * oc:128 * (oc + 1)],
                    xt_g[:, dc, :],
                    start=(dc == 0),
                    stop=(dc == DC - 1),
                )
            if oc < DC:  # q rows: scale * psum + scale * q_bias
                nc.scalar.activation(
                    qkt[:, oc, :], pq[:], Ident, bias=qb[:, oc:oc + 1], scale=SCALE
                )
            else:  # k rows: plain copy/cast
                nc.vector.tensor_copy(qkt[:, oc, :], pq[:])

        for ii in range(2):
            item = 2 * g + ii

            # ---- v in natural layout, augmented with ones column ----
            # v_aug column 64 is the all-ones column (softmax denominators land
            # on PSUM partition 64); columns 0..63 hold the per-head v values.
            vats = []
            for ci, (p0, pr) in enumerate(CHUNKS):
                vat = vap.tile([128, H, DH + 1], bf16, tag=f"va{ci}")
                nc.vector.memset(vat[0:pr, :, DH:DH + 1], 1.0)
                for s in range(2):
                    pv = ps_v.tile([128, VS], f32, tag="vps")
                    for dc in range(DC):
                        nc.tensor.matmul(
                            pv[0:pr],
                            xt_g[:, dc, N * ii + p0:N * ii + p0 + pr],
                            wv[:, dc, VS * s:VS * (s + 1)],
                            start=(dc == 0),
                            stop=(dc == DC - 1),
                        )
                    nc.vector.tensor_tensor(
                        out=vat[0:pr, 6 * s:6 * (s + 1), 0:DH],
                        in0=pv[0:pr, :].rearrange("p (h d) -> p h d", d=DH),
                        in1=vb[0:pr, VS * s:VS * (s + 1)].rearrange(
                            "p (h d) -> p h d", d=DH
                        ),
                        op=ADD,
                    )
                vats.append(vat)

            # ---- attention per head ----
            otu = otnp.tile([DH, H, N], bf16, tag="otn")
            rcp_sb = rcpp.tile([DH + 1, H, N], f32, tag="rcps")
            rcp_rep = rcpp.tile([DH, H, N], f32, tag="rcpr")
            for h in range(H):
                hb = 64 * (h % 2)
                qc = h // 2
                kc = DC + h // 2
                po = ps_ot.tile([128, N], f32, tag="otps")
                for ci, (p0, pr) in enumerate(CHUNKS):
                    ps_ = ps_s.tile([128, N], f32, tag="sps")
                    nc.tensor.matmul(
                        ps_[0:pr],
                        qkt[hb:hb + 64, kc, N * ii + p0:N * ii + p0 + pr],
                        qkt[hb:hb + 64, qc, N * ii:N * ii + N],
                        start=True,
                        stop=True,
                    )
                    pex = pep.tile([128, N], bf16, tag="pex")
                    nc.scalar.activation(pex[0:pr], ps_[0:pr], Exp)
                    ebt = eb0 if ci == 0 else eb1
                    nc.vector.tensor_mul(pex[0:pr], pex[0:pr], ebt[0:pr, h, :])
                    nc.tensor.matmul(
                        po[0:DH + 1],
                        vats[ci][0:pr, h, :],
                        pex[0:pr],
                        start=(ci == 0),
                        stop=(ci == 1),
                    )
                nc.vector.reciprocal(rcp_sb[DH:DH + 1, h, :], po[DH:DH + 1, :])
                nc.scalar.copy(otu[0:DH, h, :], po[0:DH, :])

            # broadcast reciprocals across partitions 0..63 via a DRAM bounce
            dtmp = dramp.tile([1, H, N], f32, tag="drcp")
            nc.sync.dma_start(dtmp[:], rcp_sb[DH:DH + 1, :, :])
            dsrc = dtmp[0]
            bcast = bass.AP(
                tensor=dsrc.tensor,
                offset=dsrc.offset,
                ap=[[0, DH]] + [list(a) for a in dsrc.ap],
            )
            nc.sync.dma_start(rcp_rep[0:DH, :, :], bcast)
            # normalize all heads in one op
            nc.vector.tensor_mul(
                otu[0:DH, :, :], otu[0:DH, :, :], rcp_rep[0:DH, :, :]
            )

            # ---- reorder OT [0:64, h, n] -> OT [128(d), 6, n] ----
            ot_t = otp.tile([128, DC, N], bf16, tag="ot")
            r = otu[0:DH].rearrange("p (c two) n -> p two c n", two=2)
            nc.sync.dma_start(ot_t[0:64, :, :], r[:, 0])
            nc.sync.dma_start(ot_t[64:128, :, :], r[:, 1])

            # ---- projection ----
            for p0, pr in CHUNKS:
                ob = outp.tile([128, D], f32, tag="ob")
                for s in range(2):
                    pp = ps_pr.tile([128, VS], f32, tag="prps")
                    for dc in range(DC):
                        nc.tensor.matmul(
                            pp[0:pr],
                            ot_t[:, dc, p0:p0 + pr],
                            wp[:, dc, VS * s:VS * (s + 1)],
                            start=(dc == 0),
                            stop=(dc == DC - 1),
                        )
                    nc.vector.tensor_add(
                        ob[0:pr, VS * s:VS * (s + 1)],
                        pp[0:pr],
                        pb[0:pr, VS * s:VS * (s + 1)],
                    )
                nc.sync.dma_start(t["y"][item, p0:p0 + pr, :], ob[0:pr])


def build_program(n_items=BC, enable_asserts=False):
    nc = bacc.Bacc(
        "TRN2",
        target_bir_lowering=False,
        debug=False,
        enable_asserts=enable_asserts,
        num_devices=1,
    )
    f32 = mybir.dt.float32
    bf16 = mybir.dt.bfloat16
    t = {
        "x": nc.dram_tensor("x", [n_items, N, D], f32, kind="ExternalInput").ap(),
        "wqk": nc.dram_tensor("wqk", [128, DC, NQK], bf16, kind="ExternalInput").ap(),
        "wv": nc.dram_tensor("wv", [128, DC, D], bf16, kind="ExternalInput").ap(),
        "wp": nc.dram_tensor("wp", [128, DC, D], bf16, kind="ExternalInput").ap(),
        "qb": nc.dram_tensor("qb", [128, DC], f32, kind="ExternalInput").ap(),
        "vb": nc.dram_tensor("vb", [128, D], f32, kind="ExternalInput").ap(),
        "pb": nc.dram_tensor("pb", [128, D], f32, kind="ExternalInput").ap(),
        "eb0": nc.dram_tensor("eb0", [CH0, H, N], bf16, kind="ExternalInput").ap(),
        "eb1": nc.dram_tensor("eb1", [CH1, H, N], bf16, kind="ExternalInput").ap(),
        "idn": nc.dram_tensor("idn", [128, 128], bf16, kind="ExternalInput").ap(),
        "y": nc.dram_tensor("y", [n_items, N, D], f32, kind="ExternalOutput").ap(),
    }
    with tile.TileContext(nc) as tc:
        with ExitStack() as ctx:
            _build_body(ctx, tc, t, n_items)
    nc.compile()
    return nc


def host_constants(qkv_w, q_bias, v_bias, rel_pos_table, proj_w, proj_b, rel_index):
    qkv_w = np.asarray(qkv_w, np.float32)
    proj_w = np.asarray(proj_w, np.float32)
    q_bias = np.asarray(q_bias, np.float32)
    v_bias = np.asarray(v_bias, np.float32)
    proj_b = np.asarray(proj_b, np.float32)
    rel_pos_table = np.asarray(rel_pos_table, np.float32)
    rel_index = np.asarray(rel_index)

    wt = qkv_w.T  # [768, 2304]
    wqk = wt[:, :NQK].reshape(DC, 128, NQK).transpose(1, 0, 2).astype(BF16)
    wv = wt[:, NQK:].reshape(DC, 128, D).transpose(1, 0, 2).astype(BF16)
    wp = proj_w.T.reshape(DC, 128, D).transpose(1, 0, 2).astype(BF16)
    qb = np.ascontiguousarray((SCALE * q_bias).reshape(DC, 128).T)
    vb = np.ascontiguousarray(np.tile(v_bias[None, :], (128, 1)))
    pb = np.ascontiguousarray(np.tile(proj_b[None, :], (128, 1)))
    # bias[q, k, h] -> exp -> [h, k, q] (transposed for the S^T layout)
    ebT = np.exp(rel_pos_table[rel_index].astype(np.float64)).transpose(2, 1, 0)
    eb0 = np.ascontiguousarray(ebT[:, :CH0, :].transpose(1, 0, 2)).astype(BF16)
    eb1 = np.ascontiguousarray(ebT[:, CH0:, :].transpose(1, 0, 2)).astype(BF16)
    idn = np.eye(128, dtype=BF16)
    return {
        "wqk": wqk, "wv": wv, "wp": wp, "qb": qb, "vb": vb, "pb": pb,
        "eb0": eb0, "eb1": eb1, "idn": idn,
    }


_PROG_CACHE = {}


def get_program(n_items=BC):
    if n_items not in _PROG_CACHE:
        _PROG_CACHE[n_items] = build_program(n_items)
    return _PROG_CACHE[n_items]


def run(inputs, trace=False):
    """Run on all 8 cores. Returns (output [256,197,768] f32, exec_time_ns|None)."""
    from concourse.bass_utils import run_bass_kernel_spmd

    x = np.asarray(inputs["x"], np.float32)
    consts = host_constants(
        inputs["qkv_w"], inputs["q_bias"], inputs["v_bias"],
        inputs["rel_pos_table"], inputs["proj_w"], inputs["proj_b"],
        inputs["rel_index"],
    )
    nc = get_program(BC)
    in_maps = [
        {"x": np.ascontiguousarray(x[c * BC:(c + 1) * BC]), **consts}
        for c in range(N_CORES)
    ]
    res = run_bass_kernel_spmd(
        nc, in_maps, core_ids=list(range(N_CORES)), trace=trace
    )
    out = np.concatenate([res.results[c]["y"] for c in range(N_CORES)], axis=0)
    return out, res.exec_time_ns


def kernel(**inputs) -> np.ndarray:
    out, _ = run(inputs, trace=False)
    return out
